# revision 1
# baseline (speedup 1.0000x reference)
"""MultiHeadAttention Trainium2 kernel.

Sharding: 8 cores = 4 batches x 2 head-groups (8 heads each).
Each core computes, for its (batch b, head-group g):
  Q^T = Wq_g @ Xq^T, K^T = Wk_g @ Xk^T   (f32r matmuls, [headdim, S] layout)
  V   = Xv @ Wv_g^T                       ([S, 512] layout, +ones col, mask-scaled)
  scores^T[k,q] per head (K=64 matmuls), e = exp(s/8) on ACT (PSUM->SBUF)
  x~^T/sums via [V|1]-stationary matmul (M=65), normalize via reciprocal +
  gpsimd partition_broadcast, out^T_partial = Wo_g^T.T @ x^T.
Host sums the two head-group partials per batch and transposes back.

Mask handling: V rows and the ones column are multiplied by mask (0/1), which
masks both the attnV numerator and the softmax denominator exactly.
"""
import contextlib
import os

import numpy as np
import concourse.bass as bass  # noqa: F401
import concourse.tile as tile
from concourse import bacc, mybir
from concourse.bass_utils import run_bass_kernel_spmd

F32 = mybir.dt.float32
F32R = mybir.dt.float32r
EXP = mybir.ActivationFunctionType.Exp

B, S, DM = 4, 2048, 1024
H = 16
DK = 64
HLOC = 8              # heads per core
CW = HLOC * DK        # 512 local head dims per core
NC_CORES = 8
KT = S // 128         # 16 k-tiles
NB = S // 512         # 4 q/s blocks of 512
MT = CW // 128        # 4 m-tiles of local head dims
DT = DM // 128        # 8 contraction tiles over d_model
SCALE = 1.0 / np.sqrt(DK)

_NC = None


def _env(k, d):
    return int(os.environ.get(k, d))


def _build():
    nc = bacc.Bacc()
    xqT = nc.dram_tensor("xqT", [DM, S], F32, kind="ExternalInput")
    xkT = nc.dram_tensor("xkT", [DM, S], F32, kind="ExternalInput")
    xvT = nc.dram_tensor("xvT", [DM, S], F32, kind="ExternalInput")
    wqT = nc.dram_tensor("wqT", [DM, CW], F32, kind="ExternalInput")
    wkT = nc.dram_tensor("wkT", [DM, CW], F32, kind="ExternalInput")
    wvT = nc.dram_tensor("wvT", [DM, CW], F32, kind="ExternalInput")
    woT = nc.dram_tensor("woT", [CW, DM], F32, kind="ExternalInput")
    maskf = nc.dram_tensor("maskf", [128, KT], F32, kind="ExternalInput")
    outT = nc.dram_tensor("outT", [DM, S], F32, kind="ExternalOutput")

    with tile.TileContext(nc) as tc, contextlib.ExitStack() as ctx:
        persist = ctx.enter_context(tc.tile_pool(name="persist", bufs=1))

        # --- persistent tiles: mask, wo, Q^T/K^T slices, V ---
        m_sb = persist.tile([128, KT], F32)
        nc.sync.dma_start(m_sb[:], maskf[:])
        ones8 = persist.tile([128, HLOC], F32)
        nc.vector.memset(ones8[:], 1.0)
        warm = persist.tile([1, 1], F32)
        nc.scalar.activation(warm[:], ones8[0:1, 0:1], EXP, scale=1.0)
        q_tiles = {}   # (m, nb) -> [128, 512] f32r  (Q^T slice)
        k_tiles = {}
        for m in range(MT):
            for n in range(NB):
                q_tiles[(m, n)] = persist.tile(
                    [128, 512], F32R, tag=f"q{m}_{n}", name=f"q{m}_{n}")
                k_tiles[(m, n)] = persist.tile(
                    [128, 512], F32R, tag=f"k{m}_{n}", name=f"k{m}_{n}")
        v_sb = persist.tile([128, KT, HLOC, DK + 1], F32R, tag="v")

        # ---------------- Phase A: projections ----------------
        wq_pool = ctx.enter_context(tc.tile_pool(name="wqp", bufs=1))
        xt = ctx.enter_context(tc.tile_pool(name="xt", bufs=_env("K_XT_BUFS", 8)))
        ctxA = contextlib.ExitStack()
        with ctxA:
            wkv_pool = ctxA.enter_context(tc.tile_pool(name="wkv", bufs=1))
            psA = ctxA.enter_context(tc.tile_pool(name="psA", bufs=8, space="PSUM"))
            wq_sb = [wq_pool.tile([128, CW], F32R, tag=f"wq{k}", name=f"wq{k}")
                     for k in range(DT)]
            wk_sb = [wkv_pool.tile([128, CW], F32R, tag=f"wk{k}", name=f"wk{k}")
                     for k in range(DT)]
            wv_sb = [wq_pool.tile([128, CW], F32R, tag=f"wv{k}", name=f"wv{k}")
                     for k in range(DT)]
            wo_t = [persist.tile([128, DM], F32R, tag=f"wo{k}", name=f"wo{k}")
                    for k in range(MT)]

            def w_dma(tiles, src, k):
                nc.sync.dma_start(
                    tiles[k][:],
                    src[k * 128:(k + 1) * 128, :].bitcast(F32R))

            def dma_block(src, n, nm, wtiles=None, wsrc=None):
                tiles = [xt.tile([128, 512], F32R, tag="xt",
                                 name=f"{nm}{n}_{i}") for i in range(DT)]
                for k in range(DT):
                    if wtiles is not None:
                        w_dma(wtiles, wsrc, k)
                    nc.sync.dma_start(
                        tiles[k][:],
                        src[k * 128:(k + 1) * 128,
                            n * 512:(n + 1) * 512].bitcast(F32R))
                return tiles

            def proj_group(dst_tiles, w_sb, xts, n, m, pool, tag):
                ps = pool.tile([128, 512], F32, tag=tag, name=f"pj{n}_{m}_{tag}")
                for k in range(DT):
                    nc.tensor.matmul(
                        ps[:], w_sb[k][:, m * 128:(m + 1) * 128],
                        xts[k][:], start=(k == 0), stop=(k == DT - 1))
                nc.vector.tensor_copy(dst_tiles[(m, n)][:], ps[:])

            def proj_block(dst_tiles, w_sb, src, n, nm, wsrc=None):
                xts = dma_block(src, n, nm,
                                wtiles=w_sb if wsrc is not None else None,
                                wsrc=wsrc)
                for m in range(MT):
                    proj_group(dst_tiles, w_sb, xts, n, m, psA, "pa")

            def v_group(n, sm, xts, pool, tag):
                t = n * 4 + sm
                ps = pool.tile([128, 512], F32, tag=tag, name=f"vps{n}_{sm}")
                for k in range(DT):
                    nc.tensor.matmul(
                        ps[:], xts[k][:, sm * 128:(sm + 1) * 128],
                        wv_sb[k][:], start=(k == 0), stop=(k == DT - 1))
                # evacuate with mask scaling; set+mask ones column
                nc.vector.tensor_scalar_mul(
                    v_sb[:, t, :, 0:DK],
                    ps[:].rearrange("p (h d) -> p h d", h=HLOC),
                    m_sb[:, t:t + 1])
                nc.vector.tensor_scalar_mul(
                    v_sb[:, t, :, DK:DK + 1], ones8[:],
                    m_sb[:, t:t + 1])

            def v_block(n, pool, tag, first=False):
                xts = dma_block(xvT, n, "xv",
                                wtiles=wv_sb if first else None,
                                wsrc=wvT if first else None)
                for sm in range(4):        # s-tiles within block
                    v_group(n, sm, xts, pool, tag)

            # PE warmup: dummy matmuls cover initial DMA latency and start
            # the HAM activity window before the first real matmul.
            dum = wq_pool.tile([128, 512], F32R, tag="dum")
            nc.vector.memset(dum[:].bitcast(F32), 0.0)
            for i in range(_env("K_WARM_MM", 8)):
                pw = psA.tile([128, 512], F32, tag="pa", name=f"warmmm{i}")
                nc.tensor.matmul(pw[:], dum[:, 0:128], dum[:],
                                 start=True, stop=True)
            proj_block(k_tiles, wk_sb, xkT, 0, "xk", wsrc=wkT)
            for n in range(1, NB):
                proj_block(k_tiles, wk_sb, xkT, n, "xk")
            proj_block(q_tiles, wq_sb, xqT, 0, "xq", wsrc=wqT)
            v_block(0, psA, "pa", first=True)
            v_block(1, psA, "pa")
            for k in range(MT):
                nc.sync.dma_start(
                    wo_t[k][:], woT[k * 128:(k + 1) * 128, :].bitcast(F32R))

        # ---------------- Phase B: attention + out-proj ----------------
        SGW = _env("K_SGW", 2)
        with tc.tile_pool(name="ev", bufs=_env("K_EV_BUFS", 3)) as ev, \
             tc.tile_pool(name="x", bufs=2) as xpool, \
             tc.tile_pool(name="small", bufs=_env("K_SMALL_BUFS", 2)) as small, \
             tc.tile_pool(name="o", bufs=2) as opool, \
             tc.tile_pool(name="psS", bufs=_env("K_PSS_BUFS", 3), space="PSUM") as psS, \
             tc.tile_pool(name="psX", bufs=_env("K_XO_BUFS", 2), space="PSUM") as psX:
            x_tiles = [xpool.tile([128, MT, 512], F32R, tag="xs",
                                  name=f"xs{i}") for i in range(2)]
            NSG = KT // SGW

            def outproj_group(oqt, m):
                x_prev = x_tiles[oqt % 2]
                po = psS.tile([128, 512], F32, tag="s", name=f"po{oqt}_{m}")
                for kk in range(MT):
                    nc.tensor.matmul(
                        po[:], wo_t[kk][:, m * 128:(m + 1) * 128],
                        x_prev[:, kk, :], start=(kk == 0), stop=(kk == MT - 1))
                o_sb = opool.tile([128, 512], F32, tag="ob")
                nc.vector.tensor_copy(o_sb[:], po[:])
                (nc.gpsimd if _env("K_OUT_GP", 0) else nc.sync).dma_start(
                    outT[m * 128:(m + 1) * 128, oqt * 512:(oqt + 1) * 512],
                    o_sb[:])

            # side-work: one psS-slot matmul group (or a DMA batch) per sg
            # step. (qt0,p0): v-blocks 2,3 (deadline: attnV eats V tile t at
            # emission slot t//SGW+1). (qt0,p1..3): late q projections n=p.
            # (qt>0,p0): out-projection of qt-1.
            xts_store = {}

            def mk_vdma(nn):
                def f():
                    xts_store[("v", nn)] = dma_block(xvT, nn, "xv")
                return ("dma", f)

            def mk_vg(nn, sm):
                return ("mm", lambda: v_group(nn, sm, xts_store[("v", nn)],
                                              psS, "s"))

            def mk_qdma(nn):
                def f():
                    xts_store[("q", nn)] = dma_block(xqT, nn, "xq")
                return ("dma", f)

            def mk_qg(nn, m):
                return ("mm", lambda: proj_group(q_tiles, wq_sb,
                                                 xts_store[("q", nn)],
                                                 nn, m, psS, "s"))

            side_work = {}
            VOFF = _env("K_VOFF", 0)
            side_work[(0, 0)] = [
                (0, mk_vdma(2)), (max(1, 2 + VOFF), mk_vdma(3)),
                (max(1, 2 + VOFF), mk_vg(2, 0)), (max(2, 3 + VOFF), mk_vg(2, 1)),
                (max(3, 4 + VOFF), mk_vg(2, 2)), (max(4, 5 + VOFF), mk_vg(2, 3)),
                (max(5, 6 + VOFF), mk_vg(3, 0)), (max(6, 7 + VOFF), mk_vg(3, 1)),
                (7 if VOFF < 0 else 99, mk_vg(3, 2)), (99, mk_vg(3, 3)),
            ]
            QOFF = _env("K_QOFF", 3)
            for n in range(1, NB):
                side_work[(0, n)] = [(0, mk_qdma(n))] + [
                    (QOFF + m, mk_qg(n, m)) for m in range(MT)]

            OSPREAD = _env("K_OSPREAD", 4)

            def side_step(qt, p, sg):
                if qt > 0 and p < OSPREAD:
                    per = DT // OSPREAD
                    step = (KT // SGW) // per
                    off = _env("K_OOFF", 1) + (p % 2) * _env("K_OSTAG", 0)
                    if sg % step == off:
                        outproj_group(qt - 1, p * per + sg // step)
                    return
                work = side_work.get((qt, p))
                if not work:
                    return
                did_mm = False
                while work:
                    min_sg, (kind, fn) = work[0]
                    if min_sg > sg or (kind == "mm" and did_mm):
                        break
                    work.pop(0)
                    fn()
                    if kind == "mm":
                        did_mm = True

            def side_flush(qt, p):
                for _, (kind, fn) in side_work.pop((qt, p), []):
                    fn()

            for qt in range(NB):
                x_sb = x_tiles[qt % 2]
                for p in range(MT):        # head pairs; pair p = heads 2p,2p+1
                    heads = (2 * p, 2 * p + 1)
                    ps_x = {h: psX.tile([65, 512], F32, tag="xo",
                                        name=f"psx{qt}_{h}") for h in heads}
                    e_prev = None
                    for sg in range(NSG):
                        ps_s = {h: psS.tile([128, SGW, 512], F32, tag="s",
                                            name=f"pss{qt}_{sg}_{h}")
                                for h in heads}
                        # side work: outproj of qt-1, or late q projection
                        side_step(qt, p, sg)
                        for tt in range(SGW):
                            t = sg * SGW + tt
                            for h in heads:
                                hp = h % 2
                                nc.tensor.matmul(
                                    ps_s[h][:, tt, :],
                                    k_tiles[(p, t // 4)][
                                        hp * 64:(hp + 1) * 64,
                                        (t % 4) * 128:(t % 4 + 1) * 128],
                                    q_tiles[(p, qt)][hp * 64:(hp + 1) * 64, :],
                                    start=True, stop=True)
                        # attnV for the PREVIOUS supergroup (1-sg software lag)
                        if e_prev is not None:
                            psg = sg - 1
                            if _env("K_V_ILV", 0):
                                for tt in range(SGW):
                                    t = psg * SGW + tt
                                    for h in heads:
                                        nc.tensor.matmul(
                                            ps_x[h][:], v_sb[:, t, h, :],
                                            e_prev[h][:, tt, :],
                                            start=(t == 0), stop=(t == KT - 1))
                            else:
                                for h in heads:
                                    for tt in range(SGW):
                                        t = psg * SGW + tt
                                        nc.tensor.matmul(
                                            ps_x[h][:], v_sb[:, t, h, :],
                                            e_prev[h][:, tt, :],
                                            start=(t == 0), stop=(t == KT - 1))
                        e_prev = {}
                        for h in heads:
                            e_sb = ev.tile([128, SGW, 512], F32R, tag="e",
                                           name=f"e{qt}_{sg}_{h}")
                            if _env("K_COPY_EXP", 0):
                                nc.vector.tensor_copy(e_sb[:], ps_s[h][:])
                            else:
                                nc.scalar.activation(e_sb[:], ps_s[h][:], EXP,
                                                     scale=float(SCALE))
                            e_prev[h] = e_sb
                    side_flush(qt, p)
                    last_pair = (qt == NB - 1 and p == MT - 1)
                    for h in heads:            # drain last supergroup + norm
                        psg = NSG - 1
                        for tt in range(SGW):
                            t = psg * SGW + tt
                            nc.tensor.matmul(
                                ps_x[h][:], v_sb[:, t, h, :],
                                e_prev[h][:, tt, :],
                                start=(t == 0), stop=(t == KT - 1))
                        hp = h % 2
                        if last_pair:
                            xr = ps_x[h]   # no next pair: read PSUM directly
                        else:
                            xr = small.tile([65, 512], F32, tag="xr")
                            nc.vector.tensor_copy(xr[:], ps_x[h][:])
                        r = small.tile([1, 512], F32, tag="r",
                                       name=f"r{qt}_{h}")
                        if _env("K_FAST_RECIP", 0):
                            nc.vector.reciprocal_approx_fast(r[:], xr[64:65, :])
                        else:
                            nc.vector.reciprocal(r[:], xr[64:65, :])
                        rb = small.tile([64, 512], F32, tag="rb",
                                        name=f"rb{qt}_{h}")
                        nc.gpsimd.partition_broadcast(rb[:], r[:])
                        meng = nc.gpsimd if _env("K_MUL_GP", 0) else nc.vector
                        if hp == 0:
                            meng.tensor_mul(
                                x_sb[0:64, p, :], xr[0:64, :], rb[:])
                        else:
                            xtmp = small.tile([64, 512], F32R, tag="xr", name=f"xtmp{qt}_{h}")
                            meng.tensor_mul(
                                xtmp[:], xr[0:64, :], rb[:])
                            (nc.gpsimd if _env("K_SHIFT_GP", 0)
                             else nc.sync).dma_start(
                                x_sb[64:128, p, :], xtmp[:])
            for m in range(DT):
                outproj_group(NB - 1, m)
    nc.finalize()
    return nc


def kernel(query, key, value, mask, W_q, W_k, W_v, W_o):
    global _NC
    if _NC is None:
        _NC = _build()
    query = np.asarray(query, dtype=np.float32)
    key = np.asarray(key, dtype=np.float32)
    value = np.asarray(value, dtype=np.float32)
    W_q = np.asarray(W_q, dtype=np.float32)
    W_k = np.asarray(W_k, dtype=np.float32)
    W_v = np.asarray(W_v, dtype=np.float32)
    W_o = np.asarray(W_o, dtype=np.float32)
    mask = np.asarray(mask)

    in_maps = []
    for c in range(NC_CORES):
        b, g = divmod(c, 2)
        hs = slice(g * CW, (g + 1) * CW)
        mrow = (mask[b, 0, 0, :] != 0).astype(np.float32)
        in_maps.append({
            "xqT": np.ascontiguousarray(query[b].T),
            "xkT": np.ascontiguousarray(key[b].T),
            "xvT": np.ascontiguousarray(value[b].T),
            "wqT": np.ascontiguousarray(W_q[hs, :].T),
            "wkT": np.ascontiguousarray(W_k[hs, :].T),
            "wvT": np.ascontiguousarray(W_v[hs, :].T),
            "woT": np.ascontiguousarray(W_o[:, hs].T),
            "maskf": np.ascontiguousarray(mrow.reshape(KT, 128).T),
        })
    res = run_bass_kernel_spmd(_NC, in_maps, core_ids=list(range(NC_CORES)))
    out = np.empty((B, S, DM), np.float32)
    for b in range(B):
        out[b] = (res.results[2 * b]["outT"] + res.results[2 * b + 1]["outT"]).T
    return out



# revision 22
# speedup vs baseline: 1.0519x; 1.0519x over previous
"""MultiHeadAttention Trainium2 kernel (v2: bf16 + flipped attnV + batched DMA).

Sharding: 8 cores = 4 batches x 2 head-groups (8 heads each).
Per core (batch b, head-group g):
  Q^T = Wq_g @ Xq^T, K^T = Wk_g @ Xk^T   (bf16 matmuls, f32r [headdim, S] out)
  V   = Xv @ Wv_g^T                       (bf16 [S, 8h, 65] layout, ones col,
                                           mask-scaled)
  scores^T[k,q] per head (K=64 f32r matmuls), e = exp(s/8) on ACT -> bf16
  attnV FLIPPED: x^T[q, d] psum tiles [128q, 4qs, 128] accumulated with
    e-stationary / V-moving bf16 matmuls (65-row ap => 4x fewer PE cycles)
  normalize: strided reciprocal of sums col + per-partition-scalar muls
  x^T -> x via one DMA transpose per head pair ([128q, 512] bf16)
  out^T_partial = Wo_g^T.T @ x (bf16), one output DMA per q block
Host sums the two head-group partials per batch and transposes back.
All DMAs are batched into single multi-dim-AP transfers (dispatch costs
~650ns of SEQ + HWDGE each, so count matters more than bytes).
"""
import contextlib
import os

import numpy as np
import ml_dtypes
import concourse.bass as bass  # noqa: F401
import concourse.tile as tile
from concourse import bacc, mybir
from concourse.bass_utils import run_bass_kernel_spmd

F32 = mybir.dt.float32
F32R = mybir.dt.float32r
BF16 = mybir.dt.bfloat16
NPBF = ml_dtypes.bfloat16
EXP = mybir.ActivationFunctionType.Exp
COPY = mybir.ActivationFunctionType.Copy

B, S, DM = 4, 2048, 1024
H = 16
DK = 64
HLOC = 8              # heads per core
CW = HLOC * DK        # 512 local head dims per core
NC_CORES = 8
KT = S // 128         # 16 k-tiles
NB = S // 512         # 4 q/s blocks of 512
MT = CW // 128        # 4 m-tiles of local head dims
DT = DM // 128        # 8 contraction tiles over d_model
QS = 4                # 128-wide q sub-tiles per 512 block
SCALE = 1.0 / np.sqrt(DK)

_NC = None


def _env(k, d):
    return int(os.environ.get(k, d))


def _build():
    nc = bacc.Bacc()
    xqT = nc.dram_tensor("xqT", [DM, S], BF16, kind="ExternalInput")
    xkT = nc.dram_tensor("xkT", [DM, S], BF16, kind="ExternalInput")
    xvT = nc.dram_tensor("xvT", [DM, S], BF16, kind="ExternalInput")
    wqT = nc.dram_tensor("wqT", [DM, CW], BF16, kind="ExternalInput")
    wkT = nc.dram_tensor("wkT", [DM, CW], BF16, kind="ExternalInput")
    wvT = nc.dram_tensor("wvT", [DM, CW], BF16, kind="ExternalInput")
    woT = nc.dram_tensor("woT", [CW, DM], BF16, kind="ExternalInput")
    maskf = nc.dram_tensor("maskf", [128, KT], F32, kind="ExternalInput")
    outT = nc.dram_tensor("outT", [DM, S], F32, kind="ExternalOutput")

    with tile.TileContext(nc) as tc, contextlib.ExitStack() as ctx:
        persist = ctx.enter_context(tc.tile_pool(name="persist", bufs=1))

        # --- persistent tiles: mask, wo, Q^T/K^T slices, V ---
        m_sb = persist.tile([128, KT], F32)
        nc.sync.dma_start(m_sb[:], maskf[:])
        ones8 = persist.tile([128, HLOC], BF16)
        nc.vector.memset(ones8[:], 1.0)
        warm = persist.tile([1, 1], F32)
        nc.scalar.activation(warm[:], m_sb[0:1, 0:1], EXP, scale=1.0)
        q_tiles = {}   # (m, nb) -> [128, 512] f32r  (Q^T slice)
        k_tiles = {}
        for m in range(MT):
            for n in range(NB):
                q_tiles[(m, n)] = persist.tile(
                    [128, 512], F32R, tag=f"q{m}_{n}", name=f"q{m}_{n}")
                k_tiles[(m, n)] = persist.tile(
                    [128, 512], F32R, tag=f"k{m}_{n}", name=f"k{m}_{n}")
        v_sb = persist.tile([128, KT, HLOC, DK + 1], BF16, tag="v")

        # ---------------- Phase A: projections ----------------
        wq_pool = ctx.enter_context(tc.tile_pool(name="wqp", bufs=1))
        xt = ctx.enter_context(tc.tile_pool(name="xt", bufs=_env("K_XT_BUFS", 4)))
        ctxA = contextlib.ExitStack()
        with ctxA:
            wkv_pool = ctxA.enter_context(tc.tile_pool(name="wkv", bufs=1))
            psA = ctxA.enter_context(tc.tile_pool(name="psA", bufs=8, space="PSUM"))
            # weights: one [128, DT, CW] tile per tensor, single DMA each
            wq_sb = wq_pool.tile([128, DT, CW], BF16, tag="wq")
            wk_sb = wkv_pool.tile([128, DT, CW], BF16, tag="wk")
            wv_sb = wq_pool.tile([128, DT, CW], BF16, tag="wv")
            wo_t = persist.tile([128, MT, DM], BF16, tag="wo")

            def w_dma(tile_, src):
                nc.sync.dma_start(
                    tile_[:],
                    src[:].rearrange("(k p) c -> p k c", p=128))

            def dma_block(src, n, nm):
                t = xt.tile([128, DT, 512], BF16, tag="xt", name=f"{nm}{n}")
                nc.sync.dma_start(
                    t[:],
                    src[:, n * 512:(n + 1) * 512].rearrange(
                        "(k p) c -> p k c", p=128))
                return t

            def proj_group(dst_tiles, w_sb, xts, n, m, pool, tag, gp=False):
                ps = pool.tile([128, 512], F32, tag=tag, name=f"pj{n}_{m}_{tag}")
                for k in range(DT):
                    nc.tensor.matmul(
                        ps[:], w_sb[:, k, m * 128:(m + 1) * 128],
                        xts[:, k, :], start=(k == 0), stop=(k == DT - 1))
                nc.vector.tensor_copy(dst_tiles[(m, n)][:], ps[:])

            def proj_block(dst_tiles, w_sb, src, n, nm):
                xts = dma_block(src, n, nm)
                for m in range(MT):
                    proj_group(dst_tiles, w_sb, xts, n, m, psA, "pa")

            def v_group(n, sm, xts, pool, tag):
                t = n * 4 + sm
                ps = pool.tile([128, 512], F32, tag=tag, name=f"vps{n}_{sm}")
                for k in range(DT):
                    nc.tensor.matmul(
                        ps[:], xts[:, k, sm * 128:(sm + 1) * 128],
                        wv_sb[:, k, :], start=(k == 0), stop=(k == DT - 1))
                # evacuate with mask scaling (on ACT: idle during phase A/qt0)
                if _env("K_VE_ACT", 1):
                    nc.scalar.activation(
                        v_sb[:, t, :, 0:DK],
                        ps[:].rearrange("p (h d) -> p h d", h=HLOC),
                        COPY, scale=m_sb[:, t:t + 1])
                else:
                    nc.vector.tensor_scalar_mul(
                        v_sb[:, t, :, 0:DK],
                        ps[:].rearrange("p (h d) -> p h d", h=HLOC),
                        m_sb[:, t:t + 1])
                nc.vector.tensor_scalar_mul(
                    v_sb[:, t, :, DK:DK + 1], ones8[:],
                    m_sb[:, t:t + 1])

            def v_block(n, pool, tag):
                xts = dma_block(xvT, n, "xv")
                for sm in range(4):        # s-tiles within block
                    v_group(n, sm, xts, pool, tag)

            # Front-load all phase-A DMAs (dispatch ~650ns each; transfers
            # overlap compute), then warm up the PE while they land.
            w_dma(wk_sb, wkT)
            xk_ts = [dma_block(xkT, n, "xk") for n in range(NB)]
            w_dma(wq_sb, wqT)
            xq0_ts = dma_block(xqT, 0, "xq")
            w_dma(wv_sb, wvT)
            xv_ts = [dma_block(xvT, n, "xv") for n in range(2)]
            nc.sync.dma_start(
                wo_t[:], woT[:].rearrange("(k p) c -> p k c", p=128))
            dum = wq_pool.tile([128, 512], F32R, tag="dum")
            nc.vector.memset(dum[:].bitcast(F32), 0.0)
            for i in range(_env("K_WARM_MM", 12)):
                pw = psA.tile([128, 512], F32, tag="pa", name=f"warmmm{i}")
                nc.tensor.matmul(pw[:], dum[:, 0:128], dum[:],
                                 start=True, stop=True)
            # K: all m-tiles (phase B consumes k-block n at sg 2n of every
            # pair). Q: only m0 (pair p first needs q(p, 0) at pair-p start;
            # m1..3 are side work). V blocks 0,1: attnV eats 2 V tiles per
            # sg from sg2, side production adds 1/sg, so 8 must pre-exist.
            for n in range(NB):
                for m in range(MT):
                    proj_group(k_tiles, wk_sb, xk_ts[n], n, m, psA, "pa")
            proj_group(q_tiles, wq_sb, xq0_ts, 0, 0, psA, "pa")
            for n in range(2):
                for sm in range(4):
                    v_group(n, sm, xv_ts[n], psA, "pa")
            xts_store = {("q", 0): xq0_ts}

        # ---------------- Phase B: attention + out-proj ----------------
        # One supergroup = one k-tile; both heads of the pair share a single
        # [128, 2h, 512] psum slab so the scores pipeline needs only 2 ring
        # slots and the 3rd stays free for side work / out-proj.
        with tc.tile_pool(name="ev", bufs=_env("K_EV_BUFS", 3)) as ev, \
             tc.tile_pool(name="x", bufs=2) as xpool, \
             tc.tile_pool(name="small", bufs=_env("K_SMALL_BUFS", 3)) as small, \
             tc.tile_pool(name="o", bufs=2) as opool, \
             tc.tile_pool(name="psS", bufs=_env("K_PSS_BUFS", 3), space="PSUM") as psS, \
             tc.tile_pool(name="psX", bufs=_env("K_XO_BUFS", 2), space="PSUM") as psX:
            x_tiles = [xpool.tile([128, MT, QS, 128], BF16, tag="xs",
                                  name=f"xs{i}") for i in range(2)]
            o_tiles = {}
            NSG = KT

            def outproj_group(oqt, m):
                x_prev = x_tiles[oqt % 2]
                po = psS.tile([128, 512], F32, tag="s", name=f"po{oqt}_{m}")
                for kk in range(MT):
                    nc.tensor.matmul(
                        po[:], wo_t[:, kk, m * 128:(m + 1) * 128],
                        x_prev[:, kk], start=(kk == 0), stop=(kk == MT - 1))
                if m == 0:
                    o_tiles[oqt] = opool.tile([128, DT, 512], F32, tag="ob",
                                              name=f"ob{oqt}")
                nc.vector.tensor_copy(o_tiles[oqt][:, m, :], po[:])
                if m == DT - 1:
                    nc.sync.dma_start(
                        outT[:, oqt * 512:(oqt + 1) * 512].rearrange(
                            "(k p) c -> p k c", p=128),
                        o_tiles.pop(oqt)[:])

            # side-work: one psS-slot matmul group (or a DMA batch) per sg
            # step. (qt0,p0): v-blocks 2,3 (deadline: attnV eats V tile t at
            # emission slot t//SGW+1) + late q0 m-tiles. (qt0,p1..3): late q
            # projections n=p. (qt>0,p<OSPREAD): out-projection of qt-1.
            xts_store[("q", 0)] = xq0_ts

            def mk_vdma(nn):
                def f():
                    xts_store[("v", nn)] = dma_block(xvT, nn, "xv")
                return ("dma", f)

            def mk_vg(nn, sm):
                return ("mm", lambda: v_group(nn, sm, xts_store[("v", nn)],
                                              psS, "s"))

            def mk_qdma(nn):
                def f():
                    xts_store[("q", nn)] = dma_block(xqT, nn, "xq")
                return ("dma", f)

            def mk_qg(nn, m):
                return ("mm", lambda: proj_group(q_tiles, wq_sb,
                                                 xts_store[("q", nn)],
                                                 nn, m, psS, "s", gp=True))

            side_work = {}
            VOFF = _env("K_VOFF", 0)
            # v tile t is eaten by attnV at sg t+1; vg(n, sm) makes t=4n+sm
            side_work[(0, 0)] = [
                (0, mk_vdma(2)), (0, mk_vdma(3)),
                (1, mk_qg(0, 1)),
                (5 + VOFF, mk_vg(2, 0)), (6 + VOFF, mk_vg(2, 1)),
                (7 + VOFF, mk_vg(2, 2)), (8 + VOFF, mk_vg(2, 3)),
                (9 + VOFF, mk_vg(3, 0)), (10 + VOFF, mk_vg(3, 1)),
                (11 + VOFF, mk_vg(3, 2)), (12 + VOFF, mk_vg(3, 3)),
            ]
            # q-block n+1 is projected during qt n (deadline: qt n+1 start);
            # q0's m1..3 tiles during early qt0 (deadline: pair m of qt0).
            # Max ~2 groups per pair outside (0,0) to stay under ACT pace.
            side_work[(0, 1)] = [(0, mk_qdma(1)), (1, mk_qg(0, 2)),
                                 (5, mk_qg(1, 0)), (9, mk_qg(1, 1))]
            side_work[(0, 2)] = [(1, mk_qg(0, 3)), (3, mk_qg(1, 2)),
                                 (7, mk_qg(1, 3))]
            for n in range(2, NB):
                side_work[(n - 1, 0)] = [(0, mk_qdma(n)), (3, mk_qg(n, 0)),
                                         (11, mk_qg(n, 1))]
                side_work[(n - 1, 1)] = [(3, mk_qg(n, 2)), (11, mk_qg(n, 3))]

            OSPREAD = _env("K_OSPREAD", 4)

            def side_step(qt, p, sg):
                if qt > 0 and p < OSPREAD:
                    per = DT // OSPREAD
                    step = NSG // per
                    off = _env("K_OOFF", 1) + (p % 2) * _env("K_OSTAG", 0)
                    if sg % step == off:
                        outproj_group(qt - 1, p * per + sg // step)
                        return
                work = side_work.get((qt, p))
                if not work:
                    return
                did_mm = False
                while work:
                    min_sg, (kind, fn) = work[0]
                    if min_sg > sg or (kind == "mm" and did_mm):
                        break
                    work.pop(0)
                    fn()
                    if kind == "mm":
                        did_mm = True

            def side_flush(qt, p):
                for _, (kind, fn) in side_work.pop((qt, p), []):
                    fn()

            def attnv_step(ps_x, e_sb, h, t):
                """Flipped attnV for k-tile t of head h."""
                hp = h % 2
                for qs in range(QS):
                    nc.tensor.matmul(
                        ps_x[:, qs, 0:DK + 1],
                        e_sb[:, hp, qs * 128:(qs + 1) * 128],
                        v_sb[:, t, h, :],
                        start=(t == 0 and qs == 0),
                        stop=(t == KT - 1 and qs == QS - 1),
                        skip_group_check=True)

            def do_tail(pend):
                """Drain+normalize+transpose a finished pair (runs inside the
                next pair's sg0 slot to keep ACT fed across the boundary)."""
                theads, tps_x, te_last, txstg, tx_sb, tp, tqt = pend
                for h in theads:
                    attnv_step(tps_x[h], te_last, h, NSG - 1)
                    hp = h % 2
                    r = small.tile([128, QS], F32, tag="r",
                                   name=f"r{tqt}_{h}")
                    nc.vector.reciprocal(r[:], tps_x[h][:, :, DK:DK + 1])
                    for qs in range(QS):
                        nc.vector.tensor_scalar(
                            txstg[:, qs, hp * DK:(hp + 1) * DK],
                            tps_x[h][:, qs, 0:DK],
                            r[:, qs:qs + 1], None,
                            mybir.AluOpType.mult)
                nc.sync.dma_start_transpose(tx_sb[:, tp], txstg[:])

            pending = None
            for qt in range(NB):
                x_sb = x_tiles[qt % 2]
                for p in range(MT):        # head pairs; pair p = heads 2p,2p+1
                    heads = (2 * p, 2 * p + 1)
                    # x^T psum per head: [128q, QS, 128] f32 = one zero region
                    ps_x = {h: psX.tile([128, QS, 128], F32, tag="xo",
                                        name=f"psx{qt}_{h}") for h in heads}
                    xstg = small.tile([128, QS, 128], BF16, tag="xstg",
                                      name=f"xstg{qt}_{p}")
                    def scores_mm(t):
                        ps_s = psS.tile([128, 2, 512], F32, tag="s",
                                        name=f"pss{qt}_{t}_{p}")
                        for h in heads:
                            hp = h % 2
                            nc.tensor.matmul(
                                ps_s[:, hp, :],
                                k_tiles[(p, t // 4)][
                                    hp * 64:(hp + 1) * 64,
                                    (t % 4) * 128:(t % 4 + 1) * 128],
                                q_tiles[(p, qt)][hp * 64:(hp + 1) * 64, :],
                                start=True, stop=True)
                        return ps_s

                    # scores run one k-tile AHEAD of exp so ACT never waits;
                    # attnV lags exp by one k-tile.
                    e_prev = None
                    ps_cur = scores_mm(0)
                    for sg in range(NSG):
                        ps_nxt = scores_mm(sg + 1) if sg < NSG - 1 else None
                        # side work: outproj of qt-1, or late q/v projection
                        side_step(qt, p, sg)
                        # attnV for the PREVIOUS k-tile (1-sg software lag)
                        if e_prev is not None:
                            for h in heads:
                                attnv_step(ps_x[h], e_prev, h, sg - 1)
                        if sg == 0 and pending is not None:
                            do_tail(pending)
                            pending = None
                        e_prev = ev.tile([128, 2, 512], BF16, tag="e",
                                         name=f"e{qt}_{sg}_{p}")
                        nc.scalar.activation(e_prev[:], ps_cur[:], EXP,
                                             scale=float(SCALE))
                        ps_cur = ps_nxt
                    side_flush(qt, p)
                    pending = (heads, ps_x, e_prev, xstg, x_sb, p, qt)
            do_tail(pending)
            for m in range(DT):
                outproj_group(NB - 1, m)
    nc.finalize()
    return nc


def kernel(query, key, value, mask, W_q, W_k, W_v, W_o):
    global _NC
    if _NC is None:
        _NC = _build()
    query = np.asarray(query, dtype=np.float32)
    key = np.asarray(key, dtype=np.float32)
    value = np.asarray(value, dtype=np.float32)
    W_q = np.asarray(W_q, dtype=np.float32)
    W_k = np.asarray(W_k, dtype=np.float32)
    W_v = np.asarray(W_v, dtype=np.float32)
    W_o = np.asarray(W_o, dtype=np.float32)
    mask = np.asarray(mask)

    in_maps = []
    for c in range(NC_CORES):
        b, g = divmod(c, 2)
        hs = slice(g * CW, (g + 1) * CW)
        mrow = (mask[b, 0, 0, :] != 0).astype(np.float32)
        in_maps.append({
            "xqT": np.ascontiguousarray(query[b].T).astype(NPBF),
            "xkT": np.ascontiguousarray(key[b].T).astype(NPBF),
            "xvT": np.ascontiguousarray(value[b].T).astype(NPBF),
            "wqT": np.ascontiguousarray(W_q[hs, :].T).astype(NPBF),
            "wkT": np.ascontiguousarray(W_k[hs, :].T).astype(NPBF),
            "wvT": np.ascontiguousarray(W_v[hs, :].T).astype(NPBF),
            "woT": np.ascontiguousarray(W_o[:, hs].T).astype(NPBF),
            "maskf": np.ascontiguousarray(mrow.reshape(KT, 128).T),
        })
    res = run_bass_kernel_spmd(_NC, in_maps, core_ids=list(range(NC_CORES)))
    out = np.empty((B, S, DM), np.float32)
    for b in range(B):
        out[b] = (res.results[2 * b]["outT"] + res.results[2 * b + 1]["outT"]).T
    return out
